# revision 11
# baseline (speedup 1.0000x reference)
"""Trainium2 Bass kernel for nn_DCTiDCTWrapper3D.

Math: out = x + gelu(conv1x1(irfft(rfft(x, ch-axis) * Wc, ch-axis)) + b)

The rfft -> complex-filter -> irfft chain along the 1024-channel axis is a
linear map, implemented as two dense 1024x1024 matmuls with a packed real
spectrum of exactly 1024 slots (513 Re + 511 Im). The irfft basis is folded
into the 1x1 conv weights host-side (M2 = Gp @ conv_w.T):

  per (b, t):  XF^T = M1^T @ x          [1024 slots, 392 pos]  (PE, fp8 DR)
               Y    = spectral filter on slot pairs (c, c+4)   (DVE/ACT/Pool)
               E^T  = M2^T @ Y + bias   [1024 cout, 392 pos]   (PE, fp8 DR)
               out  = x + gelu(E)                              (ACT+DVE)

Layout keeps channels/frequency slots on SBUF partitions in 8 chunks of 128
for every stage; the spectral-filter multiply pairs PSUM chunk c with chunk
c+4 on identical partitions (slot 128c+p: c<4 -> Re bin, c>=4 -> Im bin with
the same bin index; slot 512 = Re bin 512 gets a 1-partition fixup).

Per pair j (chunks j and j+4), with XF copied PSUM->SBUF bf16 pair-wise:
  ta = [XF_j; XF_j4] * wr_j     tb = [XF_j; XF_j4] * wi_j
  Y_j = ta[0] - tb[1]           Y_j4 = ta[1] + tb[0]
  Y[slot512] = XF[slot512] * Wr[512]   (fixup, partition 0 of chunk 4)

The conv bias is folded into stage 2 as a 5th fp8 matmul group against a
one-hot constant rhs, so gelu needs no per-chunk bias operand and runs on
2-chunk PSUM tiles. Stage-1/2 weights load in pair-sized slices so the
first matmul starts ~2us in. All HBM I/O is minimized: x ships as fp8
(matmul rhs) + bf16 (residual), filter as 8 bf16 chunks + Wr[512] row,
output as bf16 (~21 MB/core total). Matmuls run fp8-e4m3 DoubleRow (0.5
cyc/row), fp32 accumulate.

Sharding: data-parallel over the batch dim b (16 clips / 8 cores = 2 per
core); filter + DFT/conv weights replicated.
"""

import os
import sys

import numpy as np

for _p in ("/opt/trn_rl_repo", "/root/.axon_site/_ro/trn_rl_repo"):
    if os.path.isdir(_p) and _p not in sys.path:
        sys.path.append(_p)

import ml_dtypes

import concourse.bass as bass
import concourse.mybir as mybir
import concourse.tile as tile
from concourse import bacc
from concourse.bass_utils import run_bass_kernel_spmd

B, T, C, H, W = 16, 8, 1024, 14, 14
HW = H * W            # 196
NCORES = 8
BPC = B // NCORES     # 2 samples per core
KCH = C // 128        # 8 channel/slot chunks
NPOS = BPC * HW       # 392 matmul free dim
NPAD = 400            # fp8 interleave stride (must be 16B aligned)
F32 = mybir.dt.float32
BF16 = mybir.dt.bfloat16
FP8 = mybir.dt.float8e4
NP_FP8 = ml_dtypes.float8_e4m3
NP_BF16 = ml_dtypes.bfloat16
DR = mybir.MatmulPerfMode.DoubleRow
GELU = mybir.ActivationFunctionType.Gelu


def _dft_matrices():
    """Packed rfft matrix M1 [c, slot] and packed irfft basis Gp [slot, c],
    both ortho-normalized, slot layout: 0..512 Re(bin), 512+k Im(bin k)."""
    n = np.arange(C, dtype=np.float64)
    k_re = np.arange(513, dtype=np.float64)
    k_im = np.arange(1, 512, dtype=np.float64)
    s = 1.0 / np.sqrt(C)
    M1 = np.empty((C, C), np.float64)
    M1[:, :513] = np.cos(2 * np.pi * np.outer(n, k_re) / C) * s
    M1[:, 513:] = -np.sin(2 * np.pi * np.outer(n, k_im) / C) * s
    Gp = np.empty((C, C), np.float64)
    Gp[0, :] = s
    Gp[1:512, :] = 2.0 * np.cos(2 * np.pi * np.outer(k_im, n) / C) * s
    Gp[512, :] = ((-1.0) ** n) * s
    Gp[513:, :] = -2.0 * np.sin(2 * np.pi * np.outer(k_im, n) / C) * s
    return M1, Gp


def _pack_m1(M1):
    """[c, slot] -> [128, pair, half, g, i, c'] fp8 flat [128, 8192];
    contraction row c = g*256 + i*128 + p, slot = (pair + 4*half)*128 + c'."""
    w = M1.astype(np.float32).reshape(4, 2, 128, 2, 4, 128)  # g i p h j c'
    w = w.transpose(2, 4, 3, 0, 1, 5).reshape(128, 8192)
    return np.ascontiguousarray(w).astype(NP_FP8)


def _pack_m2(M2, conv_b):
    """[slot, cout] + bias -> [128, mp, half, g(5), i, c'] fp8 flat
    [128, 10240]; g=4 is the bias group (one-hot rhs row k=0)."""
    w = np.zeros((128, 4, 2, 5, 2, 128), np.float32)
    m = M2.astype(np.float32).reshape(4, 2, 128, 4, 2, 128)  # g i p mp h c'
    w[:, :, :, 0:4] = m.transpose(2, 3, 4, 0, 1, 5)
    w[0, :, :, 4, 0, :] = conv_b.astype(np.float32).reshape(4, 2, 128)
    return np.ascontiguousarray(w.reshape(128, 10240)).astype(NP_FP8)


def _build_nc():
    nc = bacc.Bacc(
        "TRN2", target_bir_lowering=False, debug=False, num_devices=NCORES
    )
    x8_d = nc.dram_tensor("x8", [T, 128, KCH * NPOS], FP8, kind="ExternalInput").ap()
    xt_d = nc.dram_tensor("xt", [T, 128, KCH * NPOS], BF16, kind="ExternalInput").ap()
    m1_d = nc.dram_tensor("m1", [128, 4 * 2048], FP8, kind="ExternalInput").ap()
    m2_d = nc.dram_tensor("m2", [128, 4 * 2560], FP8, kind="ExternalInput").ap()
    wf_d = nc.dram_tensor("wf", [T, 128, 8 * HW], BF16, kind="ExternalInput").ap()
    w512_d = nc.dram_tensor("w512", [T, 1, HW], BF16, kind="ExternalInput").ap()
    one_d = nc.dram_tensor("one_r", [128, 2 * NPAD], FP8, kind="ExternalInput").ap()
    out_d = nc.dram_tensor("out", [T, 128, KCH * NPOS], BF16, kind="ExternalOutput").ap()

    m1_r = m1_d.rearrange("p (j r) -> p j r", j=4)
    m2_r = m2_d.rearrange("p (j r) -> p j r", j=4)

    from contextlib import ExitStack

    with tile.TileContext(nc) as tc, ExitStack() as ctx:
        const = ctx.enter_context(tc.tile_pool(name="const", bufs=1))
        x8_pool = ctx.enter_context(tc.tile_pool(name="x8", bufs=4))
        xt_pool = ctx.enter_context(tc.tile_pool(name="xt", bufs=3))
        wf_pool = ctx.enter_context(tc.tile_pool(name="wf", bufs=4))
        xb_pool = ctx.enter_context(tc.tile_pool(name="xb", bufs=2))
        tmp_pool = ctx.enter_context(tc.tile_pool(name="tmp", bufs=4))
        y_pool = ctx.enter_context(tc.tile_pool(name="y", bufs=2))
        g_pool = ctx.enter_context(tc.tile_pool(name="g", bufs=2))
        o_pool = ctx.enter_context(tc.tile_pool(name="o", bufs=2))
        ps1_pool = ctx.enter_context(tc.tile_pool(name="ps1", bufs=2, space="PSUM"))
        ps2_pool = ctx.enter_context(tc.tile_pool(name="ps2", bufs=2, space="PSUM"))

        m1_sb = const.tile([128, 4, 2, 4, 2, 128], FP8)
        m2_sb = const.tile([128, 4, 2, 5, 2, 128], FP8)
        one_sb = const.tile([128, 2, NPAD], FP8)
        warm = const.tile([1, 16], BF16)

        def stage2_mp(t, y_sb, xt_sb, g_big, mp):
            ps = ps2_pool.tile([128, 2, 512], F32, tag="ps2")
            for h in range(2):
                for g in range(5):
                    rhs = (
                        one_sb[:, :, :NPOS]
                        if g == 4
                        else y_sb[:, 2 * g : 2 * g + 2, :NPOS]
                    )
                    nc.tensor.matmul(
                        ps[:, h, :NPOS],
                        m2_sb[:, mp, h, g, :, :],
                        rhs,
                        start=(g == 0),
                        stop=(g == 4),
                        perf_mode=DR,
                    )
            nc.scalar.activation(
                g_big[:, 2 * mp : 2 * mp + 2, :], ps[:, :, :NPOS], GELU
            )
            if mp == 3:
                # residual over all 8 chunks, then one store; the DMA is
                # issued from the DVE queue so its wait is already satisfied
                o_sb = o_pool.tile([128, KCH, BPC, HW], BF16, tag="o")
                nc.vector.tensor_add(
                    o_sb[:, :, :, :],
                    xt_sb[:, :, :, :],
                    g_big[:, :, :].rearrange("p k (b s) -> p k b s", b=BPC),
                )
                nc.scalar.dma_start(
                    out_d[t].rearrange("p (k b s) -> p k b s", k=KCH, b=BPC),
                    o_sb[:, :, :, :],
                )

        pending = None
        for t in range(T):
            if t == 0:
                # one-hot rhs first (tiny), then warm both ACT tables with
                # dep-free dummies so the loads overlap the head DMAs
                nc.sync.dma_start(one_sb[:, :, :], one_d.rearrange("p (i s) -> p i s", i=2))
                nc.scalar.activation(warm[:, :], one_sb[0:1, 0, :16], GELU)
                nc.scalar.copy(warm[:, :], one_sb[0:1, 0, :16])
                nc.sync.dma_start(m1_sb[:, 0], m1_r[:, 0])
            x8t = x8_pool.tile([128, KCH, NPAD], FP8, tag="x8")
            nc.sync.dma_start(
                x8t[:, :, :NPOS], x8_d[t].rearrange("p (k s) -> p k s", k=KCH)
            )
            wf_sb = wf_pool.tile([128, 8, HW], BF16, tag="wf")
            nc.sync.dma_start(wf_sb[:, :, :], wf_d[t].rearrange("p (k s) -> p k s", k=8))
            w512_sb = wf_pool.tile([1, HW], BF16, tag="w512")
            nc.sync.dma_start(w512_sb[:, :], w512_d[t])
            if t == 0:
                for j in range(1, 4):
                    nc.sync.dma_start(m1_sb[:, j], m1_r[:, j])
            xt_sb = xt_pool.tile([128, KCH, BPC, HW], BF16, tag="xt")
            nc.sync.dma_start(
                xt_sb[:, :, :, :],
                xt_d[t].rearrange("p (k b s) -> p k b s", k=KCH, b=BPC),
            )
            if t == 0:
                for j in range(4):
                    nc.sync.dma_start(m2_sb[:, j], m2_r[:, j])

            # pair-slot order: slot 2j = chunk j (Re), slot 2j+1 = chunk j+4
            xball = xb_pool.tile([128, 4, 2, BPC, HW], BF16, tag="xb")
            ta = tmp_pool.tile([128, 4, 2, BPC, HW], BF16, tag="ta")
            tb = tmp_pool.tile([128, 4, 2, BPC, HW], BF16, tag="tb")
            y_sb = y_pool.tile([128, KCH, NPAD], FP8, tag="y")
            g_big = g_pool.tile([128, KCH, NPOS], BF16, tag="g")

            # stage 1 (fp8 DoubleRow, K=256 per pass) + spectral filter.
            # Pair j handles chunks (j, j+4); stage2(t-1) mo-pairs interleave
            # so the PE has work while the PSUM->SBUF copies drain.
            for j in range(4):
                ps = ps1_pool.tile([128, 2, 512], F32, tag="ps1")
                for h in range(2):
                    for g in range(4):
                        nc.tensor.matmul(
                            ps[:, h, :NPOS],
                            m1_sb[:, j, h, g, :, :],
                            x8t[:, 2 * g : 2 * g + 2, :NPOS],
                            start=(g == 0),
                            stop=(g == 3),
                            perf_mode=DR,
                        )
                # PSUM pair -> SBUF bf16 (pair slots 2j, 2j+1)
                xbv = xball[:, j, :, :, :]
                nc.scalar.copy(
                    xbv.rearrange("p h b s -> p h (b s)"), ps[:, :, :NPOS]
                )
                # filter for this pair right away (overlaps later pairs)
                wr_v = wf_sb[:, j : j + 1, :].rearrange("p j (u s) -> p j u s", u=1)
                wi_v = wf_sb[:, 4 + j : 5 + j, :].rearrange("p j (u s) -> p j u s", u=1)
                nc.vector.tensor_mul(
                    ta[:, j, :, :, :], *bass.broadcast_tensor_aps(xbv, wr_v)
                )
                nc.vector.tensor_mul(
                    tb[:, j, :, :, :], *bass.broadcast_tensor_aps(xbv, wi_v)
                )
                # Y_j (Re' = Re*wr - Im*wi): j<2 on DVE, j>=2 on Pool
                seng = nc.vector if j < 2 else nc.gpsimd
                seng.tensor_sub(
                    y_sb[:, j, :NPOS],
                    ta[:, j, 0, :, :].rearrange("p b s -> p (b s)"),
                    tb[:, j, 1, :, :].rearrange("p b s -> p (b s)"),
                )
                # Y_{j+4} (Im*wr + Re*wi) on Pool
                nc.gpsimd.tensor_add(
                    y_sb[:, j + 4, :NPOS],
                    ta[:, j, 1, :, :].rearrange("p b s -> p (b s)"),
                    tb[:, j, 0, :, :].rearrange("p b s -> p (b s)"),
                )
                if j == 0:
                    # slot 512 = Re bin 512: Y = XF * Wr[512], same engine as
                    # the chunk-4 add -> ordered overwrite of partition 0
                    nc.gpsimd.tensor_mul(
                        y_sb[0:1, 4, :NPOS].rearrange("p (b s) -> p b s", b=BPC),
                        *bass.broadcast_tensor_aps(
                            xball[0:1, 0, 1, :, :],
                            w512_sb[0:1, :].rearrange("p (u s) -> p u s", u=1),
                        ),
                    )
                # stage2 of t-1 rides between pairs to keep PE busy
                if pending is not None:
                    stage2_mp(*pending, j)

            pending = (t, y_sb, xt_sb, g_big)
        # drain last t
        for mp in range(4):
            stage2_mp(*pending, mp)

    nc.compile()
    return nc


_CACHE = {}


def _get_compiled():
    if "nc" not in _CACHE:
        _CACHE["nc"] = _build_nc()
    return _CACHE["nc"]


def _host_prep(wfilt, conv_w, conv_b):
    M1, Gp = _dft_matrices()
    M2 = Gp @ conv_w.astype(np.float64).T
    m1_np = _pack_m1(M1)
    m2_np = _pack_m2(M2, conv_b)

    # filter tensor: [t, 128, 8, hw] bf16; chunks 0-3 = Wr rows 0..511,
    # chunks 4-7 = Wi rows 0..511 (row 0 zeroed); w512 = Wr[512] row
    Wr = wfilt[..., 0].reshape(T, HW, 513).transpose(0, 2, 1)  # [t, bin, hw]
    Wi = wfilt[..., 1].reshape(T, HW, 513).transpose(0, 2, 1)
    wf = np.empty((T, 128, 8, HW), np.float32)
    wf[:, :, 0:4] = Wr[:, :512].reshape(T, 4, 128, HW).transpose(0, 2, 1, 3)
    wi_rows = Wi[:, :512].copy()
    wi_rows[:, 0] = 0.0
    wf[:, :, 4:8] = wi_rows.reshape(T, 4, 128, HW).transpose(0, 2, 1, 3)
    wf_np = np.ascontiguousarray(wf.reshape(T, 128, 8 * HW)).astype(NP_BF16)
    w512_np = np.ascontiguousarray(Wr[:, 512:513, :]).astype(NP_BF16)

    one_np = np.zeros((128, 2, NPAD), np.float32)
    one_np[0, 0, :] = 1.0
    one_np = one_np.reshape(128, 2 * NPAD).astype(NP_FP8)
    return m1_np, m2_np, wf_np, w512_np, one_np


def _run(x, wfilt, conv_w, conv_b, n_segments, **spmd_kwargs):
    assert int(n_segments) == T and x.shape == (B * T, C, H, W)
    x = np.ascontiguousarray(x, dtype=np.float32)
    m1_np, m2_np, wf_np, w512_np, one_np = _host_prep(
        np.asarray(wfilt, np.float32),
        np.asarray(conv_w, np.float32),
        np.asarray(conv_b, np.float32),
    )
    # [b, t, c, h, w] -> [core, T, 128, kc, bpc, hw]
    xa = np.ascontiguousarray(
        x.reshape(NCORES, BPC, T, KCH, 128, HW).transpose(0, 2, 4, 3, 1, 5)
    )
    xt_np = xa.astype(NP_BF16).reshape(NCORES, T, 128, KCH * NPOS)
    x8_np = xa.astype(NP_FP8).reshape(NCORES, T, 128, KCH * NPOS)

    nc = _get_compiled()
    in_maps = [
        {
            "x8": x8_np[i],
            "xt": xt_np[i],
            "m1": m1_np,
            "m2": m2_np,
            "wf": wf_np,
            "w512": w512_np,
            "one_r": one_np,
        }
        for i in range(NCORES)
    ]
    res = run_bass_kernel_spmd(nc, in_maps, list(range(NCORES)), **spmd_kwargs)
    out = np.stack([r["out"] for r in res.results], axis=0)  # [8, T, 128, kc*b*hw]
    full = np.ascontiguousarray(
        out.reshape(NCORES, T, 128, KCH, BPC, HW)
        .transpose(0, 4, 1, 3, 2, 5)
        .reshape(B * T, C, H, W)
        .astype(np.float32)
    )
    return full, res


def kernel(x, wfilt, conv_w, conv_b, n_segments):
    return _run(x, wfilt, conv_w, conv_b, n_segments)[0]
